# revision 13
# baseline (speedup 1.0000x reference)
"""NT-Xent / InfoNCE loss on 8 Trainium2 NeuronCores (Bass/Tile).

Problem: h = concat(h_i, h_j) [8192, 256]; sim = h@h.T / 0.5;
loss = mean_r( logsumexp_{c != r}(sim[r, :]) - sim[r, (r+B) mod N] ).

Strategy (row-parallel, no collectives):
- Host pre-scales h by sqrt(2) (folds the 1/T=2 into the matmul), casts to
  bf16, transposes to [D, N], and feeds core c a copy whose columns are
  rotated by -c*1024.  Rotation makes the self-sim diagonal land at columns
  [bi*128, bi*128+128) and the positive-pair diagonal at 4096 + bi*128 for
  every core -- the SPMD program is identical, only the data differs.
- Each core computes its 1024 rows of sim in [128, 1024] PSUM groups
  (4 matmuls, K=256 in two 128-chunks), masks the self column (add -1e6 at
  the diagonal block), takes the group max on VectorE (negated, feeding the
  exp bias directly), and runs exp-with-accumulate on ScalarE, which yields
  the shifted group sum without any explicit reduction over the exp values.
- Per 128-row tile the 8 group (max, sum) pairs combine into a row
  logsumexp (exact two-level logsumexp: no overflow for any input).
- The positive logit is extracted from PSUM with one multiply-by-identity
  reduce.  Per-core partial sums reduce across partitions with a ones-vector
  matmul; the host adds the 8 scalars and divides by N.
"""

import numpy as np
import ml_dtypes

B = 4096
D = 256
N = 2 * B
NCORES = 8
SLAB = N // NCORES            # 1024 rows per core
P = 128                       # partitions
GW = 1024                     # psum group width (2 banks)
NG = N // GW                  # 8 groups per row-tile
NBI = SLAB // P               # 8 row-tiles per core
MASKVAL = -1.0e6

_nc_cache = None


def _build_nc():
    import concourse.bass as bass
    import concourse.bacc as bacc
    import concourse.tile as tile
    from concourse import mybir
    from concourse.masks import make_identity

    f32 = mybir.dt.float32
    bf16 = mybir.dt.bfloat16
    AX = mybir.AxisListType.X
    OP = mybir.AluOpType
    AF = mybir.ActivationFunctionType

    nc = bacc.Bacc(
        "TRN2", target_bir_lowering=False, debug=False, num_devices=NCORES,
    )
    hq = nc.dram_tensor("hq", [D, N], bf16, kind="ExternalInput")
    out = nc.dram_tensor("partial", [1, 1], f32, kind="ExternalOutput")

    with tile.TileContext(nc) as tc:
        with (
            tc.tile_pool(name="weights", bufs=1) as wpool,
            tc.tile_pool(name="const", bufs=1) as cpool,
            tc.tile_pool(name="scratch", bufs=2) as spool,
            tc.tile_pool(name="stats", bufs=2) as stpool,
            tc.tile_pool(name="small", bufs=4) as smpool,
            tc.tile_pool(name="psum", bufs=3, space="PSUM") as pspool,
            tc.tile_pool(name="psum_fin", bufs=1, space="PSUM") as psfin,
        ):
            # ---- load hq.T halves into SBUF, 4 column segments each ----
            NSEG = 4
            SEGW = N // NSEG
            hT = [
                wpool.tile([P, NSEG, SEGW], bf16, tag=f"hT{k}", name=f"hT{k}")
                for k in range(2)
            ]
            for k in range(2):
                for seg in range(NSEG):
                    nc.sync.dma_start(
                        out=hT[k][:, seg, :],
                        in_=hq[k * P:(k + 1) * P, seg * SEGW:(seg + 1) * SEGW],
                    )

            def hslice(k, c0, width):
                """[P, width] slice of K-half k covering columns c0:c0+width."""
                seg = c0 // SEGW
                off = c0 - seg * SEGW
                assert off + width <= SEGW
                return hT[k][:, seg, off:off + width]

            # ---- constants ----
            # identity pair for the diagonal self-sim mask, applied as an
            # extra accumulating matmul:  Ib.T @ negIb = MASKVAL * I
            Ib = cpool.tile([P, P], bf16)
            make_identity(nc, Ib)
            negIb = cpool.tile([P, P], bf16)
            nc.gpsimd.memset(negIb, 0.0)
            nc.gpsimd.affine_select(
                out=negIb, in_=negIb, compare_op=OP.not_equal, fill=MASKVAL,
                base=0, pattern=[[-1, P]], channel_multiplier=1,
            )
            posI = cpool.tile([P, P], f32)
            make_identity(nc, posI)
            ones = cpool.tile([P, 1], f32)
            nc.vector.memset(ones, 1.0)
            scrP = cpool.tile([P, P], f32)
            scr8 = cpool.tile([P, NG], f32)

            # ---- per-core row-tile stats (live across whole kernel) ----
            S8 = cpool.tile([P, NBI], f32)     # combined shifted sums per row-tile
            NM8 = cpool.tile([P, NBI], f32)    # -M (negated row max-of-group-max)
            POS8 = cpool.tile([P, NBI], f32)   # positive logits

            for bi in range(NBI):
                ng8 = stpool.tile([P, NG], f32, tag="ng8")    # -max per group
                cs8 = stpool.tile([P, NG], f32, tag="cs8")    # shifted sum per group
                for g in range(NG):
                    ps = pspool.tile([P, GW], f32, tag="ps")
                    for c in range(GW // 512):
                        col = g * GW + c * 512
                        # self-sim mask rides along as a third accumulating
                        # matmul in the chunk holding column bi*128
                        masked = g == 0 and c == (bi * P) // 512
                        for k in range(2):
                            nc.tensor.matmul(
                                ps[:, c * 512:(c + 1) * 512],
                                hslice(k, bi * P, P),
                                hslice(k, col, 512),
                                start=(k == 0),
                                stop=(k == 1),
                            )
                            if masked and k == 0:
                                # subset accumulate mid-group; k1 closes the
                                # group for every element of the chunk
                                nc.tensor.matmul(
                                    ps[:, bi * P:bi * P + P],
                                    Ib,
                                    negIb,
                                    start=False,
                                    stop=False,
                                    skip_group_check=True,
                                )
                    if g == NG // 2:
                        # positive pair: diagonal of block at 4096 + bi*128
                        nc.vector.scalar_tensor_tensor(
                            out=scrP,
                            in0=ps[:, bi * P:(bi + 1) * P],
                            scalar=0.0,
                            in1=posI,
                            op0=OP.bypass,
                            op1=OP.mult,
                            accum_out=POS8[:, bi:bi + 1],
                        )
                    nc.vector.tensor_reduce(
                        out=ng8[:, g:g + 1], in_=ps, axis=AX, op=OP.max, negate=True,
                    )
                    scr = spool.tile([P, GW], bf16, tag="scr")
                    nc.scalar.activation(
                        out=scr, in_=ps, func=AF.Exp,
                        bias=ng8[:, g:g + 1], scale=1.0,
                        accum_out=cs8[:, g:g + 1],
                    )
                # combine 8 groups -> row logsumexp pieces
                nm = smpool.tile([P, 1], f32, tag="nm")
                nc.vector.tensor_reduce(out=nm, in_=ng8, axis=AX, op=OP.min)
                w8 = smpool.tile([P, NG], f32, tag="w8")
                # w_g = exp(c_g - M) = exp(-ng8 + nm)
                nc.scalar.activation(out=w8, in_=ng8, func=AF.Exp, bias=nm, scale=-1.0)
                nc.vector.scalar_tensor_tensor(
                    out=scr8, in0=cs8, scalar=0.0, in1=w8,
                    op0=OP.bypass, op1=OP.mult,
                    accum_out=S8[:, bi:bi + 1],
                )
                nc.vector.tensor_copy(NM8[:, bi:bi + 1], nm)

            # ---- lse = -NM8 + log(S8); partial = sum(lse - POS8) ----
            lg8 = cpool.tile([P, NBI], f32)
            nc.scalar.activation(out=lg8, in_=S8, func=AF.Ln)
            t8 = cpool.tile([P, NBI], f32)
            nc.vector.scalar_tensor_tensor(
                out=t8, in0=lg8, scalar=0.0, in1=NM8,
                op0=OP.bypass, op1=OP.subtract,
            )
            acc = cpool.tile([P, 1], f32)
            scrA = cpool.tile([P, NBI], f32)
            nc.vector.scalar_tensor_tensor(
                out=scrA, in0=t8, scalar=0.0, in1=POS8,
                op0=OP.bypass, op1=OP.subtract,
                accum_out=acc,
            )
            # partition reduce via ones-matmul (f32)
            fin = psfin.tile([1, 1], f32)
            nc.tensor.matmul(fin, acc, ones, start=True, stop=True)
            res = cpool.tile([1, 1], f32)
            nc.vector.tensor_copy(res, fin)
            nc.sync.dma_start(out=out[:, :], in_=res)

    nc.compile()
    return nc


LAST_RESULTS = None


def kernel(h_i, h_j, batch_size):
    global _nc_cache, LAST_RESULTS
    from concourse.bass_utils import run_bass_kernel_spmd

    assert int(batch_size) == B
    h = np.concatenate([np.asarray(h_i), np.asarray(h_j)], axis=0).astype(np.float32)
    hq = (np.float32(np.sqrt(2.0)) * h).astype(ml_dtypes.bfloat16)
    hqT = np.ascontiguousarray(hq.T)                      # [D, N]
    in_maps = []
    for c in range(NCORES):
        in_maps.append({"hq": np.ascontiguousarray(np.roll(hqT, -c * SLAB, axis=1))})

    if _nc_cache is None:
        _nc_cache = _build_nc()

    res = run_bass_kernel_spmd(_nc_cache, in_maps, core_ids=list(range(NCORES)))
    LAST_RESULTS = res
    total = np.float64(0.0)
    for r in res.results:
        total += np.float64(r["partial"][0, 0])
    return np.float32(total / N)


# revision 16
# speedup vs baseline: 1.0460x; 1.0460x over previous
"""NT-Xent / InfoNCE loss on 8 Trainium2 NeuronCores (Bass/Tile).

Problem: h = concat(h_i, h_j) [8192, 256]; sim = h@h.T / 0.5;
loss = mean_r( logsumexp_{c != r}(sim[r, :]) - sim[r, (r+B) mod N] ).

Strategy (row-parallel, no collectives):
- Host pre-scales h by sqrt(2) (folds 1/T=2 into the matmul), casts to
  fp16, transposes to [D, N], and feeds core c a copy whose columns are
  rotated by -c*1024.  The rotation makes the self-sim diagonal land at
  columns [bi*128, +128) and the positive-pair diagonal at 4096 + bi*128
  for every core: the SPMD program is identical, only data differs.
- Each core computes its 1024 rows of sim in [128, 2048] PSUM groups
  (weight-reuse-ordered fp16 matmuls, K=256 in two 128-chunks; the self
  column is masked by a third accumulating matmul Ib.T @ (-60000*Ib)).
- One fused VectorE tensor_scalar per group stages sim to SBUF fp16 AND
  computes the group max via its reduce accumulator; this frees the PSUM
  slot without ScalarE in the lifecycle, so PE/DVE ping-pong at depth 2.
- ScalarE then runs ONE 8192-wide exp per 128-row tile from SBUF with
  bias = -(row max) and its sum accumulator: s_r = sum exp(sim - M_r)
  directly (exact logsumexp shift — safe for any input).
- lse = M + log(s); positives are extracted from PSUM with one
  multiply-by-identity scalar_tensor_tensor reduce.  Per-core partials
  reduce across partitions with a ones matmul; host sums 8 scalars / N.
"""

import numpy as np
import ml_dtypes

B = 4096
D = 256
N = 2 * B
NCORES = 8
SLAB = N // NCORES            # 1024 rows per core
P = 128                       # partitions
GW = 2048                     # psum group width (4 banks)
NG = N // GW                  # 4 groups per row-tile
NBI = SLAB // P               # 8 row-tiles per core
MASKVAL = -60000.0            # fp16-safe; exp(mask - M) == 0

_nc_cache = None


def _build_nc():
    import concourse.bass as bass
    import concourse.bacc as bacc
    import concourse.tile as tile
    from concourse import mybir
    from concourse.masks import make_identity

    f32 = mybir.dt.float32
    f16 = mybir.dt.float16
    bf16 = mybir.dt.bfloat16
    AX = mybir.AxisListType.X
    OP = mybir.AluOpType
    AF = mybir.ActivationFunctionType

    nc = bacc.Bacc(
        "TRN2", target_bir_lowering=False, debug=False, num_devices=NCORES,
    )
    hq = nc.dram_tensor("hq", [D, N], f16, kind="ExternalInput")
    out = nc.dram_tensor("partial", [1, 1], f32, kind="ExternalOutput")

    with tile.TileContext(nc) as tc:
        with (
            tc.tile_pool(name="weights", bufs=1) as wpool,
            tc.tile_pool(name="const", bufs=1) as cpool,
            tc.tile_pool(name="stage", bufs=2) as stpool,
            tc.tile_pool(name="scratch", bufs=1) as scpool,
            tc.tile_pool(name="stats", bufs=2) as gpool,
            tc.tile_pool(name="small", bufs=4) as smpool,
            tc.tile_pool(name="psum", bufs=2, space="PSUM") as pspool,
        ):
            # ---- constants (before DMAs so setup overlaps the loads) ----
            Ib = cpool.tile([P, P], f16)
            make_identity(nc, Ib)
            negIb = cpool.tile([P, P], f16)
            nc.gpsimd.memset(negIb, 0.0)
            nc.gpsimd.affine_select(
                out=negIb, in_=negIb, compare_op=OP.not_equal, fill=MASKVAL,
                base=0, pattern=[[-1, P]], channel_multiplier=1,
            )
            posI = cpool.tile([P, P], f32)
            make_identity(nc, posI)
            ones = cpool.tile([P, 1], f32)
            nc.vector.memset(ones, 1.0)
            scrP = cpool.tile([P, P], f32)
            scrA = cpool.tile([P, NBI], f32)

            # ---- per-core row-tile stats (live across whole kernel) ----
            S8 = cpool.tile([P, NBI], f32)     # sum exp(sim - M) per row-tile
            NM8 = cpool.tile([P, NBI], f32)    # -M (negated row max)
            POS8 = cpool.tile([P, NBI], f32)   # positive logits

            # ---- load hq halves into SBUF, 8 column segments each ----
            NSEG = 8
            SEGW = N // NSEG
            hT = [
                wpool.tile([P, NSEG, SEGW], f16, tag=f"hT{k}", name=f"hT{k}")
                for k in range(2)
            ]
            for seg in range(NSEG):
                for k in range(2):
                    nc.sync.dma_start(
                        out=hT[k][:, seg, :],
                        in_=hq[k * P:(k + 1) * P, seg * SEGW:(seg + 1) * SEGW],
                    )

            def hslice(k, c0, width):
                seg = c0 // SEGW
                off = c0 - seg * SEGW
                assert off + width <= SEGW
                return hT[k][:, seg, off:off + width]

            for bi in range(NBI):
                st = stpool.tile([P, N], f16, tag="st")
                gm = gpool.tile([P, NG], f32, tag="gm")
                for g in range(NG):
                    ps = pspool.tile([P, GW], f32, tag="ps")
                    # k-outer: one weight per 4-chunk sweep, mask rides in
                    # group 0 between the sweeps (mid-accumulation subset)
                    for k in range(2):
                        for c in range(GW // 512):
                            col = g * GW + c * 512
                            nc.tensor.matmul(
                                ps[:, c * 512:(c + 1) * 512],
                                hslice(k, bi * P, P),
                                hslice(k, col, 512),
                                start=(k == 0),
                                stop=(k == 1),
                            )
                        if k == 0 and g == 0:
                            nc.tensor.matmul(
                                ps[:, bi * P:bi * P + P],
                                Ib,
                                negIb,
                                start=False,
                                stop=False,
                                skip_group_check=True,
                            )
                    if g == NG // 2:
                        # positive pair: diagonal of block at 4096 + bi*128
                        nc.vector.scalar_tensor_tensor(
                            out=scrP,
                            in0=ps[:, bi * P:(bi + 1) * P],
                            scalar=0.0,
                            in1=posI,
                            op0=OP.bypass,
                            op1=OP.mult,
                            accum_out=POS8[:, bi:bi + 1],
                        )
                    # fused: stage to fp16 SBUF + group max accumulator
                    nc.vector.tensor_scalar(
                        out=st[:, g * GW:(g + 1) * GW],
                        in0=ps,
                        scalar1=0.0,
                        scalar2=None,
                        op0=OP.add,
                        op1=OP.max,
                        accum_out=gm[:, g:g + 1],
                    )
                nc.vector.tensor_reduce(
                    out=NM8[:, bi:bi + 1], in_=gm, axis=AX, op=OP.max, negate=True,
                )
                scr = scpool.tile([P, N], bf16, tag="scr")
                nc.scalar.activation(
                    out=scr, in_=st, func=AF.Exp,
                    bias=NM8[:, bi:bi + 1], scale=1.0,
                    accum_out=S8[:, bi:bi + 1],
                )

            # ---- lse = -NM8 + log(S8); partial = sum(lse - POS8) ----
            lg8 = cpool.tile([P, NBI], f32)
            nc.scalar.activation(out=lg8, in_=S8, func=AF.Ln)
            t8 = cpool.tile([P, NBI], f32)
            nc.vector.scalar_tensor_tensor(
                out=t8, in0=lg8, scalar=0.0, in1=NM8,
                op0=OP.bypass, op1=OP.subtract,
            )
            acc = cpool.tile([P, 1], f32)
            nc.vector.scalar_tensor_tensor(
                out=scrA, in0=t8, scalar=0.0, in1=POS8,
                op0=OP.bypass, op1=OP.subtract,
                accum_out=acc,
            )
            # partition reduce via ones-matmul (f32); reuse a psum slot
            fin = pspool.tile([P, GW], f32, tag="ps", name="fin")
            nc.tensor.matmul(fin[0:1, 0:1], acc, ones, start=True, stop=True)
            res = cpool.tile([1, 1], f32)
            nc.vector.tensor_copy(res, fin[0:1, 0:1])
            nc.sync.dma_start(out=out[:, :], in_=res)

    nc.compile()
    return nc


LAST_RESULTS = None


def kernel(h_i, h_j, batch_size):
    global _nc_cache, LAST_RESULTS
    from concourse.bass_utils import run_bass_kernel_spmd

    assert int(batch_size) == B
    h = np.concatenate([np.asarray(h_i), np.asarray(h_j)], axis=0).astype(np.float32)
    hq = (np.float32(np.sqrt(2.0)) * h).astype(np.float16)
    hqT = np.ascontiguousarray(hq.T)                      # [D, N]
    in_maps = []
    for c in range(NCORES):
        in_maps.append({"hq": np.ascontiguousarray(np.roll(hqT, -c * SLAB, axis=1))})

    if _nc_cache is None:
        _nc_cache = _build_nc()

    res = run_bass_kernel_spmd(_nc_cache, in_maps, core_ids=list(range(NCORES)))
    LAST_RESULTS = res
    total = np.float64(0.0)
    for r in res.results:
        total += np.float64(r["partial"][0, 0])
    return np.float32(total / N)


# revision 19
# speedup vs baseline: 1.0467x; 1.0007x over previous
"""NT-Xent / InfoNCE loss on 8 Trainium2 NeuronCores (Bass/Tile).

Problem: h = concat(h_i, h_j) [8192, 256]; sim = h@h.T / 0.5;
loss = mean_r( logsumexp_{c != r}(sim[r, :]) - sim[r, (r+B) mod N] ).

Strategy (row-parallel, no collectives):
- Host pre-scales h by sqrt(2) (folds 1/T=2 into the matmul), casts to
  fp16, transposes to [D, N], and feeds core c a copy whose columns are
  rotated by -c*1024.  The rotation makes the self-sim diagonal land at
  columns [bi*128, +128) and the positive-pair diagonal at 4096 + bi*128
  for every core: the SPMD program is identical, only data differs.
- Each core computes its 1024 rows of sim in [128, 2048] PSUM groups
  (weight-reuse-ordered fp16 matmuls, K=256 in two 128-chunks; the self
  column is masked by a third accumulating matmul Ib.T @ (-60000*Ib)).
- One fused VectorE tensor_scalar per group stages sim to SBUF fp16 AND
  computes the group max via its reduce accumulator; this frees the PSUM
  slot without ScalarE in the lifecycle, so PE/DVE ping-pong at depth 2.
- ScalarE then runs ONE 8192-wide exp per 128-row tile from SBUF with
  bias = -(row max) and its sum accumulator: s_r = sum exp(sim - M_r)
  directly (exact logsumexp shift — safe for any input).
- lse = M + log(s); positives are extracted from PSUM with one
  multiply-by-identity scalar_tensor_tensor reduce.  Per-core partials
  reduce across partitions with a ones matmul; host sums 8 scalars / N.
"""

import numpy as np
import ml_dtypes

B = 4096
D = 256
N = 2 * B
NCORES = 8
SLAB = N // NCORES            # 1024 rows per core
P = 128                       # partitions
GW = 2048                     # psum group width (4 banks)
NG = N // GW                  # 4 groups per row-tile
NBI = SLAB // P               # 8 row-tiles per core
MASKVAL = -60000.0            # fp16-safe; exp(mask - M) == 0

_nc_cache = None


def _build_nc():
    import concourse.bass as bass
    import concourse.bacc as bacc
    import concourse.tile as tile
    from concourse import mybir

    f32 = mybir.dt.float32
    f16 = mybir.dt.float16
    bf16 = mybir.dt.bfloat16
    AX = mybir.AxisListType.X
    OP = mybir.AluOpType
    AF = mybir.ActivationFunctionType

    nc = bacc.Bacc(
        "TRN2", target_bir_lowering=False, debug=False, num_devices=NCORES,
    )
    hq = nc.dram_tensor("hq", [D, N], f16, kind="ExternalInput")
    ib_d = nc.dram_tensor("ib", [P, P], f16, kind="ExternalInput")
    negib_d = nc.dram_tensor("negib", [P, P], f16, kind="ExternalInput")
    posi_d = nc.dram_tensor("posi", [P, P], f32, kind="ExternalInput")
    out = nc.dram_tensor("partial", [1, 1], f32, kind="ExternalOutput")

    with tile.TileContext(nc) as tc:
        with (
            tc.tile_pool(name="weights", bufs=1) as wpool,
            tc.tile_pool(name="const", bufs=1) as cpool,
            tc.tile_pool(name="stage", bufs=3) as stpool,
            tc.tile_pool(name="scratch", bufs=1) as scpool,
            tc.tile_pool(name="stats", bufs=2) as gpool,
            tc.tile_pool(name="small", bufs=4) as smpool,
            tc.tile_pool(name="psum", bufs=2, space="PSUM") as pspool,
        ):
            # ---- constants (host-provided identities; no GpSimd setup) ----
            Ib = cpool.tile([P, P], f16)
            nc.sync.dma_start(out=Ib, in_=ib_d[:, :])
            negIb = cpool.tile([P, P], f16)
            nc.sync.dma_start(out=negIb, in_=negib_d[:, :])
            posI = cpool.tile([P, P], f32)
            nc.sync.dma_start(out=posI, in_=posi_d[:, :])
            ones = cpool.tile([P, 1], f32)
            nc.vector.memset(ones, 1.0)
            scrP = cpool.tile([P, P], f32)
            scrA = cpool.tile([P, NBI], f32)

            # ---- per-core row-tile stats (live across whole kernel) ----
            S8 = cpool.tile([P, NBI], f32)     # sum exp(sim - M) per row-tile
            NM8 = cpool.tile([P, NBI], f32)    # -M (negated row max)
            POS8 = cpool.tile([P, NBI], f32)   # positive logits

            # ---- load hq halves into SBUF, 8 column segments each ----
            NSEG = 8
            SEGW = N // NSEG
            hT = [
                wpool.tile([P, NSEG, SEGW], f16, tag=f"hT{k}", name=f"hT{k}")
                for k in range(2)
            ]
            for seg in range(NSEG):
                for k in range(2):
                    nc.sync.dma_start(
                        out=hT[k][:, seg, :],
                        in_=hq[k * P:(k + 1) * P, seg * SEGW:(seg + 1) * SEGW],
                    )

            def hslice(k, c0, width):
                seg = c0 // SEGW
                off = c0 - seg * SEGW
                assert off + width <= SEGW
                return hT[k][:, seg, off:off + width]

            for bi in range(NBI):
                st = stpool.tile([P, N], f16, tag="st")
                gm = gpool.tile([P, NG], f32, tag="gm")
                for g in range(NG):
                    ps = pspool.tile([P, GW], f32, tag="ps")
                    # k-outer: one weight per 4-chunk sweep, mask rides in
                    # group 0 between the sweeps (mid-accumulation subset)
                    for k in range(2):
                        for c in range(GW // 512):
                            col = g * GW + c * 512
                            nc.tensor.matmul(
                                ps[:, c * 512:(c + 1) * 512],
                                hslice(k, bi * P, P),
                                hslice(k, col, 512),
                                start=(k == 0),
                                stop=(k == 1),
                            )
                        if k == 0 and g == 0:
                            nc.tensor.matmul(
                                ps[:, bi * P:bi * P + P],
                                Ib,
                                negIb,
                                start=False,
                                stop=False,
                                skip_group_check=True,
                            )
                    if g == NG // 2:
                        # positive pair: diagonal of block at 4096 + bi*128
                        nc.vector.scalar_tensor_tensor(
                            out=scrP,
                            in0=ps[:, bi * P:(bi + 1) * P],
                            scalar=0.0,
                            in1=posI,
                            op0=OP.bypass,
                            op1=OP.mult,
                            accum_out=POS8[:, bi:bi + 1],
                        )
                    # fused: stage to fp16 SBUF + group max accumulator
                    nc.vector.tensor_scalar(
                        out=st[:, g * GW:(g + 1) * GW],
                        in0=ps,
                        scalar1=0.0,
                        scalar2=None,
                        op0=OP.add,
                        op1=OP.max,
                        accum_out=gm[:, g:g + 1],
                    )
                nc.vector.tensor_reduce(
                    out=NM8[:, bi:bi + 1], in_=gm, axis=AX, op=OP.max, negate=True,
                )
                scr = scpool.tile([P, N], bf16, tag="scr")
                nc.scalar.activation(
                    out=scr, in_=st, func=AF.Exp,
                    bias=NM8[:, bi:bi + 1], scale=1.0,
                    accum_out=S8[:, bi:bi + 1],
                )

            # ---- lse = -NM8 + log(S8); partial = sum(lse - POS8) ----
            lg8 = cpool.tile([P, NBI], f32)
            nc.scalar.activation(out=lg8, in_=S8, func=AF.Ln)
            t8 = cpool.tile([P, NBI], f32)
            nc.vector.scalar_tensor_tensor(
                out=t8, in0=lg8, scalar=0.0, in1=NM8,
                op0=OP.bypass, op1=OP.subtract,
            )
            acc = cpool.tile([P, 1], f32)
            nc.vector.scalar_tensor_tensor(
                out=scrA, in0=t8, scalar=0.0, in1=POS8,
                op0=OP.bypass, op1=OP.subtract,
                accum_out=acc,
            )
            # partition reduce via ones-matmul (f32); reuse a psum slot
            fin = pspool.tile([P, GW], f32, tag="ps", name="fin")
            nc.tensor.matmul(fin[0:1, 0:1], acc, ones, start=True, stop=True)
            res = cpool.tile([1, 1], f32)
            nc.vector.tensor_copy(res, fin[0:1, 0:1])
            nc.sync.dma_start(out=out[:, :], in_=res)

    nc.compile()
    return nc


LAST_RESULTS = None


def kernel(h_i, h_j, batch_size):
    global _nc_cache, LAST_RESULTS
    from concourse.bass_utils import run_bass_kernel_spmd

    assert int(batch_size) == B
    h = np.concatenate([np.asarray(h_i), np.asarray(h_j)], axis=0).astype(np.float32)
    hq = (np.float32(np.sqrt(2.0)) * h).astype(np.float16)
    hqT = np.ascontiguousarray(hq.T)                      # [D, N]
    ib = np.eye(P, dtype=np.float16)
    negib = (MASKVAL * np.eye(P)).astype(np.float16)
    posi = np.eye(P, dtype=np.float32)
    in_maps = []
    for c in range(NCORES):
        in_maps.append({
            "hq": np.ascontiguousarray(np.roll(hqT, -c * SLAB, axis=1)),
            "ib": ib, "negib": negib, "posi": posi,
        })

    if _nc_cache is None:
        _nc_cache = _build_nc()

    res = run_bass_kernel_spmd(_nc_cache, in_maps, core_ids=list(range(NCORES)))
    LAST_RESULTS = res
    total = np.float64(0.0)
    for r in res.results:
        total += np.float64(r["partial"][0, 0])
    return np.float32(total / N)
